# revision 6
# baseline (speedup 1.0000x reference)
"""Self-contained Trainium2 kernel for nn_ConditionedConv1D (B=16, Cin=Cout=16,
T=262144, K=3, dilation=2, cond=3), data-parallel over batch on 8 NeuronCores.

Per core (2 samples):
  - The host splits each sample's time axis into 4 segments (+4-col halo) so
    2 samples x 4 segments x 16 channels fill all 128 SBUF partitions.
  - The host packs the adapter weights/bias as Wb [4,768] with the 768-axis in
    [k][ci][co] order; the device computes kern = [c;1].T @ Wb and scatters it
    into 3 block-diagonal [128,128] stationary matrices (one per tap).
  - The conv is 3 accumulating matmuls per 512-column chunk with rhs column
    offsets 0/2/4, PSUM -> SBUF copy, contiguous DMA back to DRAM.
"""

from contextlib import ExitStack

import numpy as np

import concourse.bacc as bacc
import concourse.tile as tile
from concourse import mybir
from concourse import bass_utils

B = 16
CIN = 16
COUT = 16
KSZ = 3
DIL = 2
HALO = DIL * (KSZ - 1)  # 4
COND = 3
T = 262144
T_OUT = T - HALO

N_CORES = 8
NSAMP = B // N_CORES  # 2 samples per core
NSEG = 4              # time segments per sample
SEG = T // NSEG       # 65536
TILE_COLS = 8192
NW = KSZ * CIN * COUT  # 768

_CACHE = {}


def _build():
    f32 = mybir.dt.float32
    f32r = mybir.dt.float32r
    nc = bacc.Bacc("TRN2", target_bir_lowering=False, debug=False,
                   num_devices=N_CORES)

    x_in = nc.dram_tensor("x", [NSAMP, NSEG, CIN, SEG + HALO], f32r,
                          kind="ExternalInput").ap()
    c_in = nc.dram_tensor("c", [NSAMP, COND], f32, kind="ExternalInput").ap()
    wb_in = nc.dram_tensor("Wb", [COND + 1, NW], f32, kind="ExternalInput").ap()
    y_out = nc.dram_tensor("y", [NSAMP, NSEG, COUT, SEG], f32,
                           kind="ExternalOutput").ap()

    x2d = x_in.rearrange("s j ci t -> (s j ci) t")   # [128, SEG+HALO]
    y2d = y_out.rearrange("s j co t -> (s j co) t")  # [128, SEG]

    with tile.TileContext(nc) as tc, ExitStack() as ctx:
        const_pool = ctx.enter_context(tc.tile_pool(name="const", bufs=1))
        in_pool = ctx.enter_context(tc.tile_pool(name="xin", bufs=2))
        out_pool = ctx.enter_context(tc.tile_pool(name="yout", bufs=2))
        psum_pool = ctx.enter_context(tc.tile_pool(name="psum", bufs=6, space="PSUM"))
        psum_a = ctx.enter_context(tc.tile_pool(name="psum_a", bufs=1, space="PSUM"))

        # adapter: kern[s, f] = sum_d c[s,d] Wb[d,f] + Wb[3,f]
        rhs_a = const_pool.tile([COND + 1, NW], f32)
        nc.sync.dma_start(out=rhs_a[:], in_=wb_in)
        lhsT_a = const_pool.tile([COND + 1, NSAMP], f32)
        nc.vector.memset(lhsT_a[:], 1.0)
        nc.gpsimd.dma_start(out=lhsT_a[0:COND, :], in_=c_in.rearrange("s d -> d s"))

        kern_ps = psum_a.tile([NSAMP, NW], f32)
        nc.tensor.matmul(kern_ps[:, 0:512], lhsT=lhsT_a[:], rhs=rhs_a[:, 0:512],
                         start=True, stop=True)
        nc.tensor.matmul(kern_ps[:, 512:NW], lhsT=lhsT_a[:], rhs=rhs_a[:, 512:NW],
                         start=True, stop=True)
        kern_sb = const_pool.tile([NSAMP, NW], f32)
        nc.vector.tensor_copy(kern_sb[:], kern_ps[:])

        # block-diagonal stationary weights (float32r via gpsimd DMA cast):
        # wtile[g*16+ci, k*128 + g*16+co] = kern[s(g), k*256 + ci*16 + co]
        wtile = const_pool.tile([128, KSZ * 128], f32r)
        wz = const_pool.tile([128, KSZ * 128], f32)
        nc.vector.memset(wz[:], 0.0)
        nc.gpsimd.dma_start(out=wtile[:], in_=wz[:])
        for s in range(NSAMP):
            for j in range(NSEG):
                g = s * NSEG + j
                for k in range(KSZ):
                    dst = wtile[g * 16:(g + 1) * 16,
                                k * 128 + g * 16: k * 128 + g * 16 + 16]
                    src = kern_sb[s:s + 1, k * 256:(k + 1) * 256].rearrange(
                        "p (ci co) -> p ci co", ci=CIN, co=COUT)
                    nc.gpsimd.dma_start(out=dst, in_=src)

        f32r = mybir.dt.float32r
        n_tiles = SEG // TILE_COLS
        n_chunks = TILE_COLS // 512
        for t in range(n_tiles):
            xt = in_pool.tile([128, TILE_COLS + HALO], f32r)
            nc.sync.dma_start(
                out=xt[:],
                in_=x2d[:, t * TILE_COLS: t * TILE_COLS + TILE_COLS + HALO])
            ot = out_pool.tile([128, TILE_COLS], f32)
            for q in range(n_chunks):
                ps = psum_pool.tile([128, 512], f32)
                for k in range(KSZ):
                    nc.tensor.matmul(
                        ps[:],
                        lhsT=wtile[:, k * 128:(k + 1) * 128],
                        rhs=xt[:, q * 512 + k * DIL: q * 512 + k * DIL + 512],
                        start=(k == 0), stop=(k == KSZ - 1),
                    )
                if q % 2 == 0:
                    nc.scalar.copy(ot[:, q * 512:(q + 1) * 512], ps[:])
                else:
                    nc.vector.tensor_copy(ot[:, q * 512:(q + 1) * 512], ps[:])
            nc.scalar.dma_start(out=y2d[:, t * TILE_COLS:(t + 1) * TILE_COLS],
                                in_=ot[:])

    nc.compile()
    return nc


def _get_nc():
    if "nc" not in _CACHE:
        _CACHE["nc"] = _build()
    return _CACHE["nc"]


def _pack_x(x_shard):
    out = np.zeros((NSAMP, NSEG, CIN, SEG + HALO), dtype=np.float32)
    for j in range(NSEG):
        end = min(j * SEG + SEG + HALO, T)
        out[:, j, :, : end - j * SEG] = x_shard[:, :, j * SEG:end]
    return out


def _pack_wb(W, b):
    idx_k, idx_ci, idx_co = np.meshgrid(
        np.arange(KSZ), np.arange(CIN), np.arange(COUT), indexing="ij")
    perm = (idx_co * (CIN * KSZ) + idx_ci * KSZ + idx_k).reshape(-1)
    wb = np.empty((COND + 1, NW), dtype=np.float32)
    wb[0:COND, :] = W.T[:, perm]
    wb[COND, :] = b[perm]
    return wb


def kernel(x, c, W, b, _trace=False):
    x = np.asarray(x, dtype=np.float32)
    c = np.asarray(c, dtype=np.float32)
    W = np.asarray(W, dtype=np.float32)
    b = np.asarray(b, dtype=np.float32)
    assert x.shape == (B, CIN, T) and c.shape == (B, COND)

    nc = _get_nc()
    wb = _pack_wb(W, b)
    in_maps = []
    for i in range(N_CORES):
        in_maps.append({
            "x": _pack_x(x[i * NSAMP:(i + 1) * NSAMP]),
            "c": np.ascontiguousarray(c[i * NSAMP:(i + 1) * NSAMP]),
            "Wb": wb,
        })

    res = bass_utils.run_bass_kernel_spmd(nc, in_maps, list(range(N_CORES)),
                                          trace=_trace)
    _CACHE["last_results"] = res

    y = np.empty((B, COUT, T_OUT), dtype=np.float32)
    for i in range(N_CORES):
        yc = res.results[i]["y"]  # [NSAMP, NSEG, COUT, SEG]
        yc = yc.transpose(0, 2, 1, 3).reshape(NSAMP, COUT, T)
        y[i * NSAMP:(i + 1) * NSAMP] = yc[:, :, :T_OUT]
    return y
